# revision 36
# baseline (speedup 1.0000x reference)
"""Trainium2 Bass kernel for nn_Decoder (MLP -> inverse token embedding ->
overlap-add -> channel-merge conv), data-parallel over batch on 8 NeuronCores.

Self-contained: hardcodes shapes; host-side numpy folds everything after the
first Linear+ReLU into per-channel fused matrices G (W2 -> Winv -> overlap-add
normalization -> 3-tap channel conv), and pre-transposes x to feature-major
[TC, E, C*TL*BL] fp16 so the device needs NO transposes of x at all:

    xT[e, tok] --matmul W1T--> h[Hc,tok] in PSUM
    --ACT/DVE relu+bias--> hT in SBUF --matmul G (accum over c,Hc)--> v[66,tok]
    --PE transpose--> vT[b,66] --strided adds (overlap-add)--> y[b,1056]

Perf structure:
  * HAM warm-up: dummy matmuls on a memset scratch tile from ~6us so the PE
    clock-gate (K=4/8 cold -> 8/8 after ~3.4us sustained activity) is fully
    open when the first real matmul issues.
  * startup DMAs split across both HWDGE queues (sync+scalar) with x chunk 0
    in 4 progressive pieces and c-major MLP order matching arrival order.
  * per-c interleave [mlp,mlp,fused,fused] keeps PE busy while ACT/DVE drain.
  * last chunk runs in 4 per-tl accumulation groups so drain/transpose/
    assembly/stores pipeline into the tail.

Sharding: batch 1024 -> 8 cores x 128.
"""

import numpy as np

import concourse.bacc as bacc
import concourse.mybir as mybir
from concourse.bass_utils import run_bass_kernel_spmd
from concourse.tile import TileContext

# problem shapes (hardcoded per contract)
B, C, T, E, H = 1024, 8, 32, 128, 256
SEG_LEN, SIG_LEN, NUM_SEG, STEP = 64, 1056, 32, 32
N_CORES = 8
BL = B // N_CORES          # local batch per core = 128
HC = H // 128              # H chunks = 2
TC = 8                     # t-chunks
TL = T // TC               # t per chunk = 4
CW = TL * BL               # tokens per (c, chunk) = 512
XW = C * CW                # tokens per chunk = 4096
GW = HC * C * 3 * 66       # g_sb columns
FD = mybir.dt.float32
FR = mybir.dt.float32r     # fp32 storage, FP22 multiply
FH = mybir.dt.float16

_CACHE = {}


def _host_prep(W1, b1, W2, b2, Winv, binv, Wconv, bconv):
    """Fold W2/Winv/normalization/conv into G [3var][C][H,66] and bias B[1056]."""
    counter = np.zeros(SIG_LEN, np.float64)
    for t in range(NUM_SEG):
        counter[t * STEP: t * STEP + SEG_LEN] += 1.0
    n = 1.0 / counter

    F = Winv.astype(np.float64) @ W2.astype(np.float64)          # [64, H]
    binv2 = Winv.astype(np.float64) @ b2.astype(np.float64) + binv.astype(np.float64)
    Wc = Wconv[0].astype(np.float64)                             # [C, 3]

    def n_of(var, s):
        if var == 0:
            return n[s]
        if var == 2:
            return n[992 + s]
        return 0.5

    G = np.zeros((3, C, H, 66), np.float64)
    for var in range(3):
        for c in range(C):
            for m_idx in range(66):
                for k in range(3):
                    s = m_idx + k - 2
                    if 0 <= s < SEG_LEN:
                        G[var, c, :, m_idx] += Wc[c, k] * n_of(var, s) * F[s, :]

    sig_b = np.zeros(SIG_LEN, np.float64)
    for t in range(NUM_SEG):
        sig_b[t * STEP: t * STEP + SEG_LEN] += binv2
    sig_b *= n
    Bvec = np.full(SIG_LEN, float(np.asarray(bconv).reshape(-1)[0]), np.float64)
    q = np.arange(SIG_LEN)
    for k in range(3):
        qq = q + k - 1
        valid = (qq >= 0) & (qq < SIG_LEN)
        for c in range(C):
            Bvec[valid] += Wc[c, k] * sig_b[qq[valid]]
    return G.astype(np.float32), Bvec.astype(np.float32)


def _g_col(hc, c, var):
    """Column offset of G slice (hc, c, var) inside g_sb [128, 2*8*3*66].
    c-major so the fused stage consumes g left-to-right (DMA piece order)."""
    return ((c * HC + hc) * 3 + var) * 66


def _chunk_ranges(tcix):
    # column ranges with uniform G variant; cols = tl*128 + b
    if tcix == 0:
        return [(0, 128, 0), (128, 512, 1)]       # t=0 -> var 0
    if tcix == TC - 1:
        return [(0, 384, 1), (384, 512, 2)]       # t=31 -> var 2
    return [(0, 512, 1)]


def _build_bass():
    nc = bacc.Bacc("TRN2")

    # host pre-transposed: x[tc, e, c*CW + tl*BL + b]  (feature-major)
    x = nc.dram_tensor("x", [TC, E, XW], FH, kind="ExternalInput")
    w1t = nc.dram_tensor("w1t", [E, H], FH, kind="ExternalInput")
    b1c = nc.dram_tensor("b1c", [128, HC], FD, kind="ExternalInput")
    g = nc.dram_tensor("g", [128, GW], FH, kind="ExternalInput")
    brep = nc.dram_tensor("brep", [BL, SIG_LEN], FD, kind="ExternalInput")
    ident = nc.dram_tensor("ident", [128, 128], FR, kind="ExternalInput")
    y = nc.dram_tensor("y", [BL, SIG_LEN], FD, kind="ExternalOutput")

    with TileContext(nc) as tc:
        with (
            tc.tile_pool(name="consts", bufs=1) as consts,
            tc.tile_pool(name="xt", bufs=3) as xt_pool,
            tc.tile_pool(name="ht", bufs=2) as ht_pool,
            tc.tile_pool(name="vsb", bufs=3) as vsb_pool,
            tc.tile_pool(name="big", bufs=1) as big_pool,
            tc.tile_pool(name="h_ps", bufs=4, space="PSUM") as hps_pool,
            tc.tile_pool(name="v_ps", bufs=3, space="PSUM") as vps_pool,
            tc.tile_pool(name="pe_out", bufs=1, space="PSUM") as peout_pool,
        ):
            w1t_sb = consts.tile([E, H], FH)
            b1c_sb = consts.tile([128, HC], FD)
            g_sb = consts.tile([128, GW], FH)
            ident_sb = consts.tile([128, 128], FR)
            scratch = consts.tile([128, 512], FH)
            brep_sb = big_pool.tile([BL, SIG_LEN], FD)

            V_sb = big_pool.tile([BL, T * 66], FD)      # v transposed: [b, t*66+m]
            y_sb = big_pool.tile([BL, SIG_LEN], FD)

            # ---- HAM warm-up: keep PE busy from ~6us so the clock-gate is
            # fully open (K=8/8) before the first real matmul. Dummy MMs of
            # zeros cover the ~3.4us sustained-activity window and roughly
            # bridge until the first x piece's DMA completion (~10.5us).
            nc.gpsimd.memset(scratch[:], 0.0)

            def filler(nm, n=512):
                wps = hps_pool.tile([128, 512], FD, tag="h_ps", name=nm)
                nc.tensor.matmul(wps[:, 0:n], scratch[:, 0:128],
                                 scratch[:, 0:n], start=True, stop=True)

            for wi in range(11):
                filler(f"warm_{wi}")

            # pre-warm the ACT function table during the DMA window so the
            # first relu doesn't pay the ~1.3us LoadActFuncSet
            warm = consts.tile([1, 2], FD)
            nc.gpsimd.memset(warm[:], 0.0)
            nc.scalar.activation(
                warm[:, 1:2], warm[:, 0:1],
                mybir.ActivationFunctionType.Relu, scale=1.0)

            # ---- startup DMAs. Supply model (measured): descriptors start
            # ~0.8us after their queue-gen, aggregate ~270 GB/s, FIFO within
            # a queue, round-robin across the two HWDGE queues. So: emit
            # per-channel pieces alternating queues in exact consumption
            # order -> each piece completes just before its deadline.
            xt_tiles = {}
            xt_tiles[0] = xt_pool.tile([E, XW], FH, tag="xt", name="xt_0")
            xt_tiles[1] = xt_pool.tile([E, XW], FH, tag="xt", name="xt_1")
            x0, x1 = xt_tiles[0], xt_tiles[1]

            # DMA rules (measured): each HWDGE ring holds ~512 descriptors
            # (a [128-row] DMA = 128), a gen with a full ring BLOCKS its
            # sequencer, and rings round-robin ~400 GB/s aggregate. So the
            # scalar (ACT) ring gets exactly TWO head pieces (256 desc,
            # never blocks -> relu drains flow), and everything else queues
            # on sync in consumption order where blocking gens are harmless.
            nc.scalar.dma_start(out=x0[:, 0:4 * CW], in_=x[0, :, 0:4 * CW])
            nc.scalar.dma_start(out=x0[:, 4 * CW:], in_=x[0, :, 4 * CW:])
            nc.sync.dma_start(out=w1t_sb[:], in_=w1t[:])
            nc.sync.dma_start(out=b1c_sb[:], in_=b1c[:])
            nc.sync.dma_start(out=g_sb[:, 0:GW // 2], in_=g[:, 0:GW // 2])
            nc.sync.dma_start(out=g_sb[:, GW // 2:], in_=g[:, GW // 2:])
            nc.sync.dma_start(out=x1[:], in_=x[1])
            nc.sync.dma_start(out=ident_sb[0:66, 0:66], in_=ident[0:66, 0:66])

            def emit_x_load(tcix):
                t = xt_pool.tile([E, XW], FH, tag="xt", name=f"xt_{tcix}")
                nc.sync.dma_start(out=t[:], in_=x[tcix])
                xt_tiles[tcix] = t

            # greedy ACT/DVE load balancer for PSUM->SBUF copies and relus
            eng_busy = {"act": 0.0, "dve": 0.0}

            def pick_engine(fd):
                ca = (172 + fd) / 1.2
                cd = (120 + fd) / 0.96
                if eng_busy["act"] + ca <= eng_busy["dve"] + cd:
                    eng_busy["act"] += ca
                    return "act"
                eng_busy["dve"] += cd
                return "dve"

            def bal_copy(out, in_, fd):
                if pick_engine(fd) == "act":
                    nc.scalar.copy(out=out, in_=in_)
                else:
                    nc.vector.tensor_copy(out=out, in_=in_)

            ht_tiles = {}

            def emit_mlp1(tcix, c, hc):
                """one matmul + one relu drain for (c, hc)."""
                xt = xt_tiles[tcix]
                ht = ht_tiles[tcix]
                h_ps = hps_pool.tile([128, CW], FD, tag="h_ps",
                                     name=f"h_ps_{tcix}_{hc}_{c}")
                nc.tensor.matmul(
                    h_ps[:],
                    w1t_sb[:, hc * 128:(hc + 1) * 128],
                    xt[:, c * CW:(c + 1) * CW],
                    start=True, stop=True,
                )
                dst = ht[(c // 2, hc)][:, (c % 2) * CW:(c % 2 + 1) * CW]
                if pick_engine(CW) == "act":
                    nc.scalar.activation(
                        dst, h_ps[:],
                        mybir.ActivationFunctionType.Relu,
                        bias=b1c_sb[:, hc:hc + 1], scale=1.0,
                    )
                else:
                    nc.vector.tensor_scalar(
                        dst, h_ps[:],
                        b1c_sb[:, hc:hc + 1], 0.0,
                        mybir.AluOpType.add, mybir.AluOpType.max,
                    )

            def emit_fused(tcix, v_tiles, c, hc, i, n_acc):
                """fused G matmul (c, hc) accumulating into v_tiles ranges."""
                ht = ht_tiles[tcix]
                hsrc = ht[(c // 2, hc)]
                off = (c % 2) * CW
                for (lo, hi, var, v_ps) in v_tiles:
                    nc.tensor.matmul(
                        v_ps[:, lo:hi],
                        g_sb[:, _g_col(hc, c, var):_g_col(hc, c, var) + 66],
                        hsrc[:, off + lo:off + hi],
                        start=(i == 0), stop=(i == n_acc - 1),
                    )

            def emit_vdrain(tcix, v_tiles):
                """copy v psum -> sbuf; emitted early so the ACT/DVE queue
                clears it before the PE reaches the transposes."""
                del ht_tiles[tcix]
                v_sb = vsb_pool.tile([66, CW], FR, tag="v_sb")
                for (lo, hi, var, v_ps) in v_tiles:
                    bal_copy(v_sb[:, lo:hi], v_ps[:, lo:hi], hi - lo)
                return v_sb

            def emit_vtrans(tcix, v_sb):
                """PE-transpose per t into V_sb (one wide drain per chunk)."""
                vt_ps = peout_pool.tile([128, TL * 66], FR, tag="pe_out")
                for tl in range(TL):
                    nc.tensor.transpose(
                        vt_ps[:, tl * 66:(tl + 1) * 66],
                        v_sb[:, tl * 128:(tl + 1) * 128],
                        ident_sb[0:66, 0:66],
                    )
                bal_copy(V_sb[:, tcix * TL * 66:(tcix + 1) * TL * 66],
                         vt_ps[:], TL * 66)

            # overlap-add assembly in rounds (per watermark) so it overlaps
            # with later chunks instead of serializing at the end
            V3 = V_sb[:].rearrange("b (t m) -> b t m", m=66)
            Y3 = y_sb[:].rearrange("b (j r) -> b j r", r=32)
            B3 = brep_sb[:].rearrange("b (j r) -> b j r", r=32)

            def emit_y_assembly(j_lo, j_hi, eng=None):
                """Assemble y blocks j in [j_lo, j_hi); requires V[t] for
                t <= j_hi (uses t=j+1 for the r=31 edge)."""
                if eng is None:
                    eng = nc.gpsimd
                jm = min(j_hi, 32)      # main1 defined for j<=31
                if jm > j_lo:
                    eng.tensor_add(
                        out=Y3[:, j_lo:jm, :], in0=V3[:, j_lo:jm, 1:33],
                        in1=B3[:, j_lo:jm, :])
                if j_hi == 33:          # last block: bias only here
                    eng.tensor_copy(
                        out=y_sb[:, 1024:1056], in_=brep_sb[:, 1024:1056])
                lo = max(1, j_lo)
                if j_hi > lo:           # += v[:, j-1, r+33]
                    eng.tensor_add(
                        out=Y3[:, lo:j_hi, :], in0=Y3[:, lo:j_hi, :],
                        in1=V3[:, lo - 1:j_hi - 1, 33:65])
                lo = max(2, j_lo)
                if j_hi > lo:           # r=0: += v[:, j-2, 65]
                    eng.tensor_add(
                        out=Y3[:, lo:j_hi, 0], in0=Y3[:, lo:j_hi, 0],
                        in1=V3[:, lo - 2:j_hi - 2, 65])
                hi = min(j_hi, 31)
                if hi > j_lo:           # r=31: += v[:, j+1, 0]
                    eng.tensor_add(
                        out=Y3[:, j_lo:hi, 31], in0=Y3[:, j_lo:hi, 31],
                        in1=V3[:, j_lo + 1:hi + 1, 0])

            # rounds: after vtrans(3) -> j<15 (t<=15 avail); after vtrans(6)
            # -> j<27; blocks 27+ pipeline per-tl with the last chunk
            asm_rounds = {3: (0, 15), 6: (15, 27)}

            def do_vtrans(pend):
                """vtrans for a drained chunk + its assembly round/store."""
                emit_vtrans(pend[0], pend[1])
                if pend[0] in asm_rounds:
                    emit_y_assembly(*asm_rounds[pend[0]])
                    if pend[0] == 3:
                        # blocks j<15 final: ship the first 480 cols early
                        nc.sync.dma_start(out=y[:, 0:480], in_=y_sb[:, 0:480])
                    elif pend[0] == 6:
                        nc.scalar.dma_start(out=y[:, 480:864],
                                            in_=y_sb[:, 480:864])

            prev = None          # (tcix, v_tiles) awaiting fused stage
            pending_drain = None   # (tcix, v_tiles) fused done, needs drain
            pending_vtrans = None  # (tcix, v_sb) drained, needs transposes
            for tcix in range(TC):
                if tcix + 2 < TC:
                    emit_x_load(tcix + 2)
                if tcix == 3:
                    nc.sync.dma_start(out=brep_sb[:], in_=brep[:])
                ht_tiles[tcix] = {
                    (cp, hc): ht_pool.tile(
                        [128, 2 * CW], FH,
                        tag=f"ht{hc}_{cp}", name=f"ht_{tcix}_{hc}_{cp}")
                    for cp in range(C // 2) for hc in range(HC)}
                if pending_drain is not None:
                    # drain the chunk fused during the previous iteration now
                    # so its transposes (at c==2 below) never wait on ACT/DVE
                    pending_vtrans = (pending_drain[0],
                                      emit_vdrain(*pending_drain))
                    pending_drain = None
                if tcix == 0:
                    # x0 lands in halves; fillers pace consumption to the
                    # measured DMA supply so the PE never idles (idle ->
                    # HAM re-throttle, which is far more expensive)
                    for c in range(C):
                        for hc in range(HC):
                            emit_mlp1(tcix, c, hc)
                        for fi in range({1: 1, 2: 1, 3: 2}.get(c, 0)):
                            filler(f"pace0_{c}_{fi}")
                elif tcix == 1:
                    # g precedes x1 on the sync ring: chunk-0's fused stage
                    # runs first, chunk-1's mlp after (x1 lands meanwhile)
                    for c in range(C):
                        for hc in range(HC):
                            emit_fused(prev[0], prev[1], c, hc,
                                       c * HC + hc, C * HC)
                    for c in range(C):
                        for hc in range(HC):
                            emit_mlp1(tcix, c, hc)
                    pending_drain, prev = prev, None
                else:
                    # fused-first per-c interleave: [fused, fused, mlp, mlp]
                    # keeps the PE fed while relu copies drain PSUM and gives
                    # this chunk's x pieces ~0.9us more arrival slack. The
                    # final iteration completes the whole fused stage first so
                    # chunk-6's v drains long before the tail needs it.
                    last_it = tcix == TC - 1
                    for c in range(C):
                        for hc in range(HC):
                            emit_fused(prev[0], prev[1], c, hc,
                                       c * HC + hc, C * HC)
                        if not last_it:
                            for hc in range(HC):
                                emit_mlp1(tcix, c, hc)
                        if c == 2 and pending_vtrans is not None:
                            do_vtrans(pending_vtrans)
                            pending_vtrans = None
                    if last_it:
                        pending_vtrans = (prev[0], emit_vdrain(*prev))
                        for c in range(C):
                            for hc in range(HC):
                                emit_mlp1(tcix, c, hc)
                        prev = None
                    else:
                        pending_drain, prev = prev, None
                del xt_tiles[tcix]
                if tcix < TC - 1:
                    v_tiles = [
                        (lo, hi, var,
                         vps_pool.tile([66, CW], FD, tag="v_ps",
                                       name=f"v_ps_{tcix}_{lo}"))
                        for (lo, hi, var) in _chunk_ranges(tcix)]
                    prev = (tcix, v_tiles)

            # ---- last chunk (t=28..31): 4 per-tl accumulation groups so the
            # drain/transpose/assembly of tl k overlaps the matmuls of tl k+1,
            # and y blocks 27.. finalize + store as soon as V[28+k] lands.
            lt = TC - 1
            ht_last = ht_tiles[lt]
            v_sb = vsb_pool.tile([66, CW], FR, tag="v_sb")
            vt_ps_box = []      # allocated lazily in PSUM-buffer use order

            def asm_block(j, eng):
                """Finalize y block j (cols 32j..32j+32); needs V[t<=j+1]."""
                if j < 32:
                    eng.tensor_add(out=Y3[:, j:j + 1, :],
                                   in0=V3[:, j:j + 1, 1:33],
                                   in1=B3[:, j:j + 1, :])
                else:
                    eng.tensor_copy(out=y_sb[:, 1024:1056],
                                    in_=brep_sb[:, 1024:1056])
                eng.tensor_add(out=Y3[:, j:j + 1, :], in0=Y3[:, j:j + 1, :],
                               in1=V3[:, j - 1:j, 33:65])
                eng.tensor_add(out=Y3[:, j:j + 1, 0], in0=Y3[:, j:j + 1, 0],
                               in1=V3[:, j - 2:j - 1, 65])
                if j < 31:
                    eng.tensor_add(out=Y3[:, j:j + 1, 31],
                                   in0=Y3[:, j:j + 1, 31],
                                   in1=V3[:, j + 1:j + 2, 0])

            vps_last = {}

            def last_mms(tl):
                lo, hi = tl * 128, (tl + 1) * 128
                var = 2 if tl == TL - 1 else 1
                v_ps = vps_pool.tile([66, CW], FD, tag="v_ps",
                                     name=f"v_ps_l{tl}")
                vps_last[tl] = v_ps
                for c in range(C):
                    for hc in range(HC):
                        i = c * HC + hc
                        nc.tensor.matmul(
                            v_ps[:, 0:128],
                            g_sb[:, _g_col(hc, c, var):_g_col(hc, c, var) + 66],
                            ht_last[(c // 2, hc)][:, (c % 2) * CW + lo:
                                                  (c % 2) * CW + hi],
                            start=(i == 0), stop=(i == C * HC - 1),
                        )
                bal_copy(v_sb[:, lo:hi], v_ps[:, 0:128], 128)

            def last_trans(tl):
                """PE transpose of tl (after its drain) + V copy + assembly of
                the y block that V[28+tl] completes, then a staggered store."""
                if not vt_ps_box:
                    vt_ps_box.append(
                        peout_pool.tile([128, TL * 66], FR, tag="pe_out",
                                        name="vt_ps_last"))
                vt_ps = vt_ps_box[0]
                nc.tensor.transpose(
                    vt_ps[:, tl * 66:(tl + 1) * 66],
                    v_sb[:, tl * 128:(tl + 1) * 128],
                    ident_sb[0:66, 0:66],
                )
                bal_copy(V_sb[:, (lt * TL + tl) * 66:(lt * TL + tl + 1) * 66],
                         vt_ps[:, tl * 66:(tl + 1) * 66], 66)
                if tl < TL - 1:
                    asm_block(27 + tl, nc.vector if tl % 2 == 0 else nc.gpsimd)
                else:
                    asm_block(30, nc.vector)
                    asm_block(31, nc.gpsimd)
                    asm_block(32, nc.vector)
                if tl == 1:
                    nc.sync.dma_start(out=y[:, 864:928], in_=y_sb[:, 864:928])
                elif tl == 2:
                    nc.scalar.dma_start(out=y[:, 928:960],
                                        in_=y_sb[:, 928:960])

            # transposes trail one tl group behind the matmuls so the PE
            # never waits on an ACT/DVE drain mid-stream; chunk-6's deferred
            # vtrans (drained during chunk-7's mlp) slots in after two groups
            last_mms(0)
            last_mms(1)
            do_vtrans(pending_vtrans)
            pending_vtrans = None
            last_mms(2)
            last_trans(0)
            last_mms(3)
            last_trans(1)
            last_trans(2)
            last_trans(3)
            del ht_tiles[lt]
            nc.sync.dma_start(out=y[:, 960:SIG_LEN], in_=y_sb[:, 960:SIG_LEN])

    nc.finalize()
    return nc


def make_in_maps(inputs):
    """Per-core input maps (shared by kernel(), sim checks, and bench)."""
    x = np.asarray(inputs["encoder_output"], dtype=np.float32)
    W1 = np.asarray(inputs["W1"], np.float32)
    b1 = np.asarray(inputs["b1"], np.float32)

    G, Bvec = _host_prep(
        inputs["W1"], inputs["b1"], inputs["W2"], inputs["b2"],
        inputs["Winv"], inputs["binv"], inputs["Wconv"], inputs["bconv"])

    # pack G -> [128, HC*C*3*66]: g_sb[p, _g_col(hc,c,var)+m] = G[var, c, hc*128+p, m]
    g_pack = np.zeros((128, HC * C * 3 * 66), np.float32)
    for hc in range(HC):
        for c in range(C):
            for var in range(3):
                col = _g_col(hc, c, var)
                g_pack[:, col:col + 66] = G[var, c, hc * 128:(hc + 1) * 128, :]

    w1t = np.ascontiguousarray(W1.T).astype(np.float16)     # [E, H]
    g_pack = g_pack.astype(np.float16)
    b1c = np.ascontiguousarray(b1.reshape(HC, 128).T)       # [128, HC]
    brep = np.ascontiguousarray(np.broadcast_to(Bvec, (BL, SIG_LEN)))
    ident = np.eye(128, dtype=np.float32)

    # [B,C,T,E] -> per-shard [TC, E, C*TL*BL] fp16 (feature-major tokens)
    xh = x.astype(np.float16)
    xs = xh.reshape(N_CORES, BL, C, TC, TL, E).transpose(0, 3, 5, 2, 4, 1)
    xs = np.ascontiguousarray(xs).reshape(N_CORES, TC, E, XW)
    return [
        {
            "x": xs[i],
            "w1t": w1t, "b1c": b1c, "g": g_pack,
            "brep": brep, "ident": ident,
        }
        for i in range(N_CORES)
    ]


def kernel(**inputs) -> np.ndarray:
    if "nc" not in _CACHE:
        _CACHE["nc"] = _build_bass()
    nc = _CACHE["nc"]

    in_maps = make_in_maps(inputs)
    res = run_bass_kernel_spmd(nc, in_maps, core_ids=list(range(N_CORES)))
    _CACHE["last_result"] = res
    y = np.concatenate([r["y"] for r in res.results], axis=0)   # [B, 1056]
    return y.reshape(B, 1, SIG_LEN).astype(np.float32)


if __name__ == "__main__":
    rng = np.random.default_rng(0)
    ins = {
        "encoder_output": rng.standard_normal((B, C, T, E), dtype=np.float32),
        "W1": rng.standard_normal((H, E), dtype=np.float32) / np.sqrt(E),
        "b1": rng.standard_normal((H,), dtype=np.float32) / np.sqrt(E),
        "W2": rng.standard_normal((E, H), dtype=np.float32) / np.sqrt(H),
        "b2": rng.standard_normal((E,), dtype=np.float32) / np.sqrt(H),
        "Winv": rng.standard_normal((SEG_LEN, E), dtype=np.float32) / np.sqrt(E),
        "binv": rng.standard_normal((SEG_LEN,), dtype=np.float32) / np.sqrt(E),
        "Wconv": rng.standard_normal((1, C, 3), dtype=np.float32) / np.sqrt(C * 3),
        "bconv": rng.standard_normal((1,), dtype=np.float32) / np.sqrt(C * 3),
    }
    out = kernel(**ins)
    print("kernel output", out.shape, out.dtype)
